# revision 36
# baseline (speedup 1.0000x reference)
"""DropToken gather kernel for Trainium2 (8 NeuronCores).

Computes out[b, c, :] = inputs[b, idx[c], :] (the reference's one-hot
matmul is just a row gather). Memory-bound.

Shipped design (INTERLEAVE=4): ~32.7 us vs 53.7 us f32 baseline.

  * bf16 payload: inputs are cast to bf16 host-side and gathered/stored
    as bf16, halving HBM traffic to 4 MiB read + 4 MiB write per core.
    Output is cast back to f32 host-side. Max elementwise rel err ~2^-9
    (~3e-3), well inside the 2e-2 gate.
  * Batch interleaving: all 4 batches share idx, so x is uploaded as
    [8192, 4*1024] (4 batches concatenated per row) and ONE descriptor
    fetches an 8 KB row covering all batches. Each core covers 512 cap
    positions = 512 descriptors in 4 indirect ops.

    This fixes the two bottlenecks of the per-batch layout at once:
    Q7 SWDGE descriptor emission (~1.4 us per 128-descriptor op; 16 ops
    paced the f32/bf16 kernels at ~22 us) drops to 4 ops (~5.6 us, fully
    hidden), and the random reads grow from 2 KB to 8 KB, lifting the
    measured HBM aggregate to ~373 GB/s (at the per-NC wall).
  * HW semantics (probe-verified): indirect_dma_start uses only ONE
    offset per partition; per-descriptor length = the dest AP's
    per-partition extent. [128, n] offset APs do NOT gather n rows per
    partition (columns past 0 are ignored; consecutive source rows are
    streamed instead) -- hence one [128,1]-offset op per idx column.

Timeline (core 0): ~5.9 us runtime boot + ~2.9 us idx load/receipt +
~22.5 us HBM-bound gather+store (8.39 MB at ~373 GB/s) + ~1.5 us final
store receipt. The data phase sits at the per-NC HBM wall, so further
gains would need a shorter boot or cheaper idx receipt, not DMA work.

Measured dead ends (kept as flags for reference): dma_gather ucode
(InstDMAGatherAnt) pays a ~10.5 us Q7 library reload and ~10 ns/desc;
sorted indices cause HBM bank contention (+2.6 us); dual-ring stores and
gpsimd-issued idx loads are neutral-to-worse; IL=2 (4 KB descs) is ~1 us
worse than IL=4.
"""

import ml_dtypes
import numpy as np

import concourse.bass as bass
import concourse.tile as tile
from concourse import bacc, mybir
from concourse.bass_utils import run_bass_kernel_spmd

B = 4
LENGTH = 8192
EMBED = 1024
CAP = 4096
N_CORES = 8
ROWS_PER_CORE = B * CAP // N_CORES  # 2048
T = ROWS_PER_CORE // 128  # 16 gathered rows per partition

BF16 = True
# Store grouping (in T units): one SBUF tile + one store per group. Early
# groups wide (big store descriptors), tail narrow (short last chain).
GGROUPS = [4, 4, 4, 2, 1, 1]
# WIDE=True issues ONE indirect_dma_start per group with a [128, n] offset
# AP. CoreSim accepts it but HW descriptor ordering differs (wrong results +
# can wedge the device) -- keep False until the HW mapping is understood.
WIDE = False
# InstDMAGatherAnt variant: one Q7 instruction per chunk, but needs a
# ~10.5 us Q7 library reload before the first gather and emits at ~10
# ns/desc anyway -- measured slower (58.5 us) than the indirect path.
USE_DMA_GATHER = False
DG_CHUNKS = [8, 4, 2, 1, 1]
# Raw-block variant of the indirect path: dedicated semaphore per gather
# op (Tile reuses 8 DMASW lanes, which couples op N's emission to op
# N-8's DMA completion and stretches the emission cadence).
USE_RAW = True
# Batch-interleaved gather: upload x as [LENGTH, IL*EMBED] with IL batches
# concatenated per row (all batches share idx), so one descriptor fetches
# IL rows at once. IL=4: 512 descs/core in 4 ops (Q7 emission ~5.6 us,
# fully hidden) and 8 KB random reads (vs 2 KB) for better HBM efficiency.
# HW semantics probe-validated: offsets [128,1], per-desc length = dest
# partition-row bytes.
INTERLEAVE = 4
# cap positions per core and gather ops (one [128,1]-offset op + one store
# per 128 positions) follow from IL: IL=4 -> 512 pos, 4 ops of 1 MB;
# IL=2 -> 1024 pos, 8 ops of 512 KB (smaller tail store, smaller upload).
POS_PER_CORE = CAP * B // (N_CORES * INTERLEAVE)
IL_OPS = POS_PER_CORE // 128
# which engine issues the idx load ("scalar" = HWDGE ACT ring,
# "gpsimd" = SWDGE, starts earlier after boot)
IDX_ENGINE = "scalar"
# Sort each core's indices ascending (slot -> cap position is inverse-
# permuted host-side): SDMA engines then read monotonically increasing
# HBM addresses. Measured WORSE (~+2.6 us) -- clustered addresses create
# HBM bank contention across engines; random spread balances banks.
SORT_IDX = False
# Alternate stores between the SP (sync) and ACT (scalar) HWDGE rings.
# Measured neutral-to-worse (min 34.7 vs 32.8 us); stores are gather-paced
# and the bus is HBM-capped, so a second ring only adds scheduling churn.
DUAL_STORE_RING = False
STRIP_INIT_BARRIER = True

_nc_cache = None
_nc_cache_key = None


def _strip_init_barrier(nc):
    """Remove the Bass-init const memsets and all-engine barrier from the
    entry block. This kernel has no cross-engine deps besides DMA
    semaphores (runtime-zeroed at NEFF load), so engine-boot alignment is
    unnecessary; saves ~3us of startup."""
    blk = nc.m.functions[0].blocks[0]
    blk.instructions = [
        ins
        for ins in blk.instructions
        if not isinstance(
            ins, (mybir.InstMemset, mybir.InstDrain, mybir.InstEventSemaphore)
        )
    ]


def _dt():
    return mybir.dt.bfloat16 if BF16 else mybir.dt.float32


def _np_dt():
    return ml_dtypes.bfloat16 if BF16 else np.float32


def _build_nc_dma_gather():
    """Raw-block variant using InstDMAGatherAnt.

    Index layout (host-prepared, int16): desired[j] = source row for
    gathered slot j, where slot j lands in SBUF dst[j%128, j//128, :].
    The instruction reads index j from idx16[j%16, j//16] (partitions
    0-15, replicated x8 across the 128 partitions for the 8 Q7 cores).
    We want SBUF[p, c] = x[idx_flat[p*T + c]] so the store to DRAM is
    contiguous, i.e. desired = idx_flat.reshape(128, T).T.ravel().
    """
    from contextlib import ExitStack

    assert sum(DG_CHUNKS) == T
    nc = bacc.Bacc(
        "TRN2",
        target_bir_lowering=False,
        debug=False,
        num_devices=N_CORES,
    )
    x = nc.dram_tensor("x", [LENGTH, EMBED], _dt(), kind="ExternalInput").ap()
    idx16 = nc.dram_tensor(
        "idx16", [128, ROWS_PER_CORE // 16], mybir.dt.int16, kind="ExternalInput"
    ).ap()
    out = nc.dram_tensor(
        "out", [128, T * EMBED], _dt(), kind="ExternalOutput"
    ).ap()

    nchunks = len(DG_CHUNKS)
    with ExitStack() as ctx:
        idx_tile = ctx.enter_context(
            nc.sbuf_tensor([128, ROWS_PER_CORE // 16], mybir.dt.int16)
        )
        g = ctx.enter_context(nc.sbuf_tensor([128, T, EMBED], _dt()))
        isem = ctx.enter_context(nc.semaphore("isem"))
        ssem = ctx.enter_context(nc.semaphore("ssem"))
        gsems = [ctx.enter_context(nc.semaphore(f"gsem{i}")) for i in range(nchunks)]
        block = ctx.enter_context(nc.Block())

        @block.scalar
        def _(scalar):
            scalar.dma_start(out=idx_tile[:, :], in_=idx16[:, :]).then_inc(isem, 16)

        @block.gpsimd
        def _(gpsimd):
            gpsimd.wait_ge(isem, 16)
            c0 = 0
            for i, n in enumerate(DG_CHUNKS):
                gpsimd.dma_gather(
                    g[:, c0 : c0 + n, :],
                    x[:, :],
                    idx_tile[:, c0 * 8 : (c0 + n) * 8],
                    n * 128,
                    n * 128,
                    EMBED,
                ).then_inc(gsems[i], 16)
                c0 += n

        @block.sync
        def _(sync):
            c0 = 0
            for i, n in enumerate(DG_CHUNKS):
                sync.wait_ge(gsems[i], 16)
                sync.dma_start(
                    out=out[:, c0 * EMBED : (c0 + n) * EMBED],
                    in_=g[:, c0 : c0 + n, :],
                ).then_inc(ssem, 16)
                c0 += n
            sync.wait_ge(ssem, 16 * nchunks)

    if STRIP_INIT_BARRIER:
        _strip_init_barrier(nc)
    nc.compile()
    return nc


def _build_nc_il():
    """Batch-interleaved gather: x is [LENGTH, IL*EMBED] (IL batches per
    row), each core covers CAP/N_CORES cap positions with one 8 KB
    descriptor per position. IL_OPS ops of [128,1] offsets; store per op."""
    from contextlib import ExitStack

    ilw = INTERLEAVE * EMBED  # elems per interleaved row
    nc = bacc.Bacc(
        "TRN2",
        target_bir_lowering=False,
        debug=False,
        num_devices=N_CORES,
    )
    x = nc.dram_tensor("x", [LENGTH, ilw], _dt(), kind="ExternalInput").ap()
    idx = nc.dram_tensor(
        "idx", [128, IL_OPS], mybir.dt.int32, kind="ExternalInput"
    ).ap()
    out = nc.dram_tensor(
        "out", [128, IL_OPS * ilw], _dt(), kind="ExternalOutput"
    ).ap()

    with ExitStack() as ctx:
        idx_tile = ctx.enter_context(nc.sbuf_tensor([128, IL_OPS], mybir.dt.int32))
        g = ctx.enter_context(nc.sbuf_tensor([128, IL_OPS * ilw], _dt()))
        isem = ctx.enter_context(nc.semaphore("isem"))
        ssem = ctx.enter_context(nc.semaphore("ssem"))
        gsems = [ctx.enter_context(nc.semaphore(f"gsem{o}")) for o in range(IL_OPS)]
        block = ctx.enter_context(nc.Block())

        @block.scalar
        def _(scalar):
            if IDX_ENGINE == "scalar":
                scalar.dma_start(out=idx_tile[:, :], in_=idx[:, :]).then_inc(
                    isem, 16
                )
            if DUAL_STORE_RING:
                for o in range(1, IL_OPS, 2):
                    scalar.wait_ge(gsems[o], 16)
                    scalar.dma_start(
                        out=out[:, o * ilw : (o + 1) * ilw],
                        in_=g[:, o * ilw : (o + 1) * ilw],
                    ).then_inc(ssem, 16)

        @block.gpsimd
        def _(gpsimd):
            if IDX_ENGINE == "gpsimd":
                gpsimd.dma_start(out=idx_tile[:, :], in_=idx[:, :]).then_inc(
                    isem, 16
                )
            gpsimd.wait_ge(isem, 16)
            for o in range(IL_OPS):
                gpsimd.indirect_dma_start(
                    out=g[:, o * ilw : (o + 1) * ilw],
                    out_offset=None,
                    in_=x[:, :],
                    in_offset=bass.IndirectOffsetOnAxis(
                        ap=idx_tile[:, o : o + 1], axis=0
                    ),
                ).then_inc(gsems[o], 16)

        @block.sync
        def _(sync):
            for o in range(IL_OPS):
                if DUAL_STORE_RING and o % 2:
                    continue
                sync.wait_ge(gsems[o], 16)
                sync.dma_start(
                    out=out[:, o * ilw : (o + 1) * ilw],
                    in_=g[:, o * ilw : (o + 1) * ilw],
                ).then_inc(ssem, 16)
            sync.wait_ge(ssem, 16 * IL_OPS)

    if STRIP_INIT_BARRIER:
        _strip_init_barrier(nc)
    nc.compile()
    return nc


def _build_nc_raw():
    """Raw blocks, 16 indirect gathers each with a dedicated semaphore so
    nothing couples Q7 emission of op N to DMA completion of earlier ops.
    Stores taper per GGROUPS; store i waits only on the gathers it covers."""
    from contextlib import ExitStack

    nc = bacc.Bacc(
        "TRN2",
        target_bir_lowering=False,
        debug=False,
        num_devices=N_CORES,
    )
    x = nc.dram_tensor("x", [LENGTH, EMBED], _dt(), kind="ExternalInput").ap()
    idx = nc.dram_tensor("idx", [128, T], mybir.dt.int32, kind="ExternalInput").ap()
    out = nc.dram_tensor(
        "out", [128, T * EMBED], _dt(), kind="ExternalOutput"
    ).ap()

    assert sum(GGROUPS) == T
    with ExitStack() as ctx:
        idx_tile = ctx.enter_context(nc.sbuf_tensor([128, T], mybir.dt.int32))
        g = ctx.enter_context(nc.sbuf_tensor([128, T * EMBED], _dt()))
        isem = ctx.enter_context(nc.semaphore("isem"))
        ssem = ctx.enter_context(nc.semaphore("ssem"))
        gsems = [ctx.enter_context(nc.semaphore(f"gsem{t}")) for t in range(T)]
        block = ctx.enter_context(nc.Block())

        @block.scalar
        def _(scalar):
            scalar.dma_start(out=idx_tile[:, :], in_=idx[:, :]).then_inc(isem, 16)

        @block.gpsimd
        def _(gpsimd):
            gpsimd.wait_ge(isem, 16)
            for t in range(T):
                gpsimd.indirect_dma_start(
                    out=g[:, t * EMBED : (t + 1) * EMBED],
                    out_offset=None,
                    in_=x[:, :],
                    in_offset=bass.IndirectOffsetOnAxis(
                        ap=idx_tile[:, t : t + 1], axis=0
                    ),
                ).then_inc(gsems[t], 16)

        @block.sync
        def _(sync):
            t0 = 0
            for gw in GGROUPS:
                for j in range(gw):
                    sync.wait_ge(gsems[t0 + j], 16)
                sync.dma_start(
                    out=out[:, t0 * EMBED : (t0 + gw) * EMBED],
                    in_=g[:, t0 * EMBED : (t0 + gw) * EMBED],
                ).then_inc(ssem, 16)
                t0 += gw
            sync.wait_ge(ssem, 16 * len(GGROUPS))

    if STRIP_INIT_BARRIER:
        _strip_init_barrier(nc)
    nc.compile()
    return nc


def _build_nc():
    if INTERLEAVE > 1:
        return _build_nc_il()
    if USE_DMA_GATHER:
        return _build_nc_dma_gather()
    if USE_RAW:
        return _build_nc_raw()
    nc = bacc.Bacc(
        "TRN2",
        target_bir_lowering=False,
        debug=False,
        num_devices=N_CORES,
    )
    x = nc.dram_tensor("x", [LENGTH, EMBED], _dt(), kind="ExternalInput").ap()
    idx = nc.dram_tensor("idx", [128, T], mybir.dt.int32, kind="ExternalInput").ap()
    out = nc.dram_tensor(
        "out", [128, T * EMBED], _dt(), kind="ExternalOutput"
    ).ap()

    assert sum(GGROUPS) == T

    with tile.TileContext(nc) as tc:
        with (
            tc.tile_pool(name="idxp", bufs=1) as idxp,
            tc.tile_pool(name="io", bufs=len(GGROUPS)) as io,
        ):
            idx_tile = idxp.tile([128, T], mybir.dt.int32)
            nc.scalar.dma_start(out=idx_tile[:], in_=idx[:, :])
            gmax = max(GGROUPS)
            t0 = 0
            for gw in GGROUPS:
                g = io.tile([128, gmax * EMBED], _dt(), tag="g")
                if WIDE:
                    nc.gpsimd.indirect_dma_start(
                        out=g[:, : gw * EMBED],
                        out_offset=None,
                        in_=x[:, :],
                        in_offset=bass.IndirectOffsetOnAxis(
                            ap=idx_tile[:, t0 : t0 + gw], axis=0
                        ),
                    )
                else:
                    for j in range(gw):
                        t = t0 + j
                        nc.gpsimd.indirect_dma_start(
                            out=g[:, j * EMBED : (j + 1) * EMBED],
                            out_offset=None,
                            in_=x[:, :],
                            in_offset=bass.IndirectOffsetOnAxis(
                                ap=idx_tile[:, t : t + 1], axis=0
                            ),
                        )
                nc.sync.dma_start(
                    out=out[:, t0 * EMBED : (t0 + gw) * EMBED],
                    in_=g[:, : gw * EMBED],
                )
                t0 += gw
    if STRIP_INIT_BARRIER:
        _strip_init_barrier(nc)
    nc.compile()
    return nc


def _get_nc():
    global _nc_cache, _nc_cache_key
    key = (
        BF16,
        tuple(GGROUPS),
        WIDE,
        USE_DMA_GATHER,
        tuple(DG_CHUNKS),
        USE_RAW,
        INTERLEAVE,
        IL_OPS,
        STRIP_INIT_BARRIER,
    )
    if _nc_cache is None or _nc_cache_key != key:
        _nc_cache = _build_nc()
        _nc_cache_key = key
    return _nc_cache


def _shard_inputs(inputs: np.ndarray, idx: np.ndarray):
    in_maps = []
    if INTERLEAVE > 1:
        il = INTERLEAVE
        ngroups = B // il  # batch groups; cores split across groups
        cpg = N_CORES // ngroups
        x_ils = [
            np.ascontiguousarray(
                inputs[gi * il : (gi + 1) * il]
                .transpose(1, 0, 2)
                .reshape(LENGTH, il * EMBED)
                .astype(_np_dt())
            )
            for gi in range(ngroups)
        ]
        for k in range(N_CORES):
            gi, q = divmod(k, cpg)
            vals = idx[q * POS_PER_CORE : (q + 1) * POS_PER_CORE].astype(np.int32)
            if SORT_IDX:
                vals = np.sort(vals)
            # slot (p, o) = sorted-rank o*128 + p
            idx_t = np.ascontiguousarray(vals.reshape(IL_OPS, 128).T)
            in_maps.append({"x": x_ils[gi], "idx": idx_t})
        return in_maps
    half = CAP // 2
    for k in range(N_CORES):
        b, h = divmod(k, 2)
        idx_flat = idx[h * half : (h + 1) * half].astype(np.int32)
        xs = np.ascontiguousarray(inputs[b]).astype(_np_dt())
        if USE_DMA_GATHER:
            # desired[j] = row for gathered slot j (slot j -> dst[j%128, j//128])
            desired = idx_flat.reshape(128, T).T.ravel().astype(np.int16)
            # idx16[p, s] = desired[s*16 + p] for p in 0..15, replicated x8
            wrapped = desired.reshape(ROWS_PER_CORE // 16, 16).T  # [16, R/16]
            idx16 = np.ascontiguousarray(np.tile(wrapped, (8, 1)))
            in_maps.append({"x": xs, "idx16": idx16})
        else:
            shard = np.ascontiguousarray(idx_flat.reshape(128, T))
            in_maps.append({"x": xs, "idx": shard})
    return in_maps


def _run(inputs: np.ndarray, idx: np.ndarray, **run_kwargs):
    nc = _get_nc()
    in_maps = _shard_inputs(inputs, idx)
    res = run_bass_kernel_spmd(nc, in_maps, list(range(N_CORES)), **run_kwargs)
    out = np.empty((B, CAP, EMBED), np.float32)
    if INTERLEAVE > 1:
        il = INTERLEAVE
        cpg = N_CORES // (B // il)
        for k in range(N_CORES):
            gi, q = divmod(k, cpg)
            arr = (
                res.results[k]["out"]
                .reshape(128, IL_OPS, il, EMBED)
                .astype(np.float32)
            )
            # [p, o, j, e] -> slot rank o*128+p; rank r holds cap position
            # q*POS + order[r] (order = argsort when SORT_IDX)
            tmp = arr.transpose(2, 1, 0, 3).reshape(il, POS_PER_CORE, EMBED)
            sl = out[
                gi * il : (gi + 1) * il,
                q * POS_PER_CORE : (q + 1) * POS_PER_CORE,
            ]
            if SORT_IDX:
                vals = idx[q * POS_PER_CORE : (q + 1) * POS_PER_CORE]
                sl[:, np.argsort(vals, kind="stable")] = tmp
            else:
                sl[:] = tmp
        return out, res
    half = CAP // 2
    for k in range(N_CORES):
        b, h = divmod(k, 2)
        out[b, h * half : (h + 1) * half] = (
            res.results[k]["out"].reshape(ROWS_PER_CORE, EMBED).astype(np.float32)
        )
    return out, res


def kernel(inputs: np.ndarray, idx: np.ndarray) -> np.ndarray:
    inputs = np.asarray(inputs, dtype=np.float32)
    idx = np.asarray(idx, dtype=np.int32)
    out, _ = _run(inputs, idx)
    return out


# revision 41
# speedup vs baseline: 1.0654x; 1.0654x over previous
"""DropToken gather kernel for Trainium2 (8 NeuronCores).

Computes out[b, c, :] = inputs[b, idx[c], :] (the reference's one-hot
matmul is just a row gather). Memory-bound.

Shipped design (INTERLEAVE=4): ~32.7 us vs 53.7 us f32 baseline.

  * bf16 payload: inputs are cast to bf16 host-side and gathered/stored
    as bf16, halving HBM traffic to 4 MiB read + 4 MiB write per core.
    Output is cast back to f32 host-side. Max elementwise rel err ~2^-9
    (~3e-3), well inside the 2e-2 gate.
  * Batch interleaving: all 4 batches share idx, so x is uploaded as
    [8192, 4*1024] (4 batches concatenated per row) and ONE descriptor
    fetches an 8 KB row covering all batches. Each core covers 512 cap
    positions = 512 descriptors in 4 indirect ops.

    This fixes the two bottlenecks of the per-batch layout at once:
    Q7 SWDGE descriptor emission (~1.4 us per 128-descriptor op; 16 ops
    paced the f32/bf16 kernels at ~22 us) drops to 4 ops (~5.6 us, fully
    hidden), and the random reads grow from 2 KB to 8 KB, lifting the
    measured HBM aggregate to ~373 GB/s (at the per-NC wall).
  * HW semantics (probe-verified): indirect_dma_start uses only ONE
    offset per partition; per-descriptor length = the dest AP's
    per-partition extent. [128, n] offset APs do NOT gather n rows per
    partition (columns past 0 are ignored; consecutive source rows are
    streamed instead) -- hence one [128,1]-offset op per idx column.

Timeline (core 0): ~5.9 us runtime boot + ~2.9 us idx load/receipt +
~22.5 us HBM-bound gather+store (8.39 MB at ~373 GB/s) + ~1.5 us final
store receipt. The data phase sits at the per-NC HBM wall, so further
gains would need a shorter boot or cheaper idx receipt, not DMA work.

Measured dead ends (kept as flags for reference): dma_gather ucode
(InstDMAGatherAnt) pays a ~10.5 us Q7 library reload and ~10 ns/desc;
sorted indices cause HBM bank contention (+2.6 us); dual-ring stores and
gpsimd-issued idx loads are neutral-to-worse; IL=2 (4 KB descs) is ~1 us
worse than IL=4.
"""

import ml_dtypes
import numpy as np

import concourse.bass as bass
import concourse.tile as tile
from concourse import bacc, mybir
from concourse.bass_utils import run_bass_kernel_spmd

B = 4
LENGTH = 8192
EMBED = 1024
CAP = 4096
N_CORES = 8
ROWS_PER_CORE = B * CAP // N_CORES  # 2048
T = ROWS_PER_CORE // 128  # 16 gathered rows per partition

BF16 = True
# Store grouping (in T units): one SBUF tile + one store per group. Early
# groups wide (big store descriptors), tail narrow (short last chain).
GGROUPS = [4, 4, 4, 2, 1, 1]
# WIDE=True issues ONE indirect_dma_start per group with a [128, n] offset
# AP. CoreSim accepts it but HW descriptor ordering differs (wrong results +
# can wedge the device) -- keep False until the HW mapping is understood.
WIDE = False
# InstDMAGatherAnt variant: one Q7 instruction per chunk, but needs a
# ~10.5 us Q7 library reload before the first gather and emits at ~10
# ns/desc anyway -- measured slower (58.5 us) than the indirect path.
USE_DMA_GATHER = False
DG_CHUNKS = [8, 4, 2, 1, 1]
# Raw-block variant of the indirect path: dedicated semaphore per gather
# op (Tile reuses 8 DMASW lanes, which couples op N's emission to op
# N-8's DMA completion and stretches the emission cadence).
USE_RAW = True
# Batch-interleaved gather: upload x as [LENGTH, IL*EMBED] with IL batches
# concatenated per row (all batches share idx), so one descriptor fetches
# IL rows at once. IL=4: 512 descs/core in 4 ops (Q7 emission ~5.6 us,
# fully hidden) and 8 KB random reads (vs 2 KB) for better HBM efficiency.
# HW semantics probe-validated: offsets [128,1], per-desc length = dest
# partition-row bytes.
INTERLEAVE = 4
# cap positions per core and gather ops (one [128,1]-offset op + one store
# per 128 positions) follow from IL: IL=4 -> 512 pos, 4 ops of 1 MB;
# IL=2 -> 1024 pos, 8 ops of 512 KB (smaller tail store, smaller upload).
POS_PER_CORE = CAP * B // (N_CORES * INTERLEAVE)
IL_OPS = POS_PER_CORE // 128
# which engine issues the idx load ("scalar" = HWDGE ACT ring,
# "gpsimd" = SWDGE, starts earlier after boot)
IDX_ENGINE = "scalar"
# Throwaway SWDGE copy on gpsimd to warm the Q7/ring path while idx
# loads: measured worse (median 37.6 vs 32.8) -- the extra ring traffic
# costs more than the warm-up saves.
WARM_SWDGE = False
# Sort each core's indices ascending (slot -> cap position is inverse-
# permuted host-side): SDMA engines then read monotonically increasing
# HBM addresses. Measured WORSE (~+2.6 us) -- clustered addresses create
# HBM bank contention across engines; random spread balances banks.
SORT_IDX = False
# Alternate stores between the SP (sync) and ACT (scalar) HWDGE rings.
# Measured neutral-to-worse (min 34.7 vs 32.8 us); stores are gather-paced
# and the bus is HBM-capped, so a second ring only adds scheduling churn.
DUAL_STORE_RING = False
STRIP_INIT_BARRIER = True

_nc_cache = None
_nc_cache_key = None


def _strip_init_barrier(nc):
    """Remove the Bass-init const memsets and all-engine barrier from the
    entry block. This kernel has no cross-engine deps besides DMA
    semaphores (runtime-zeroed at NEFF load), so engine-boot alignment is
    unnecessary; saves ~3us of startup."""
    blk = nc.m.functions[0].blocks[0]
    blk.instructions = [
        ins
        for ins in blk.instructions
        if not isinstance(
            ins, (mybir.InstMemset, mybir.InstDrain, mybir.InstEventSemaphore)
        )
    ]


def _dt():
    return mybir.dt.bfloat16 if BF16 else mybir.dt.float32


def _np_dt():
    return ml_dtypes.bfloat16 if BF16 else np.float32


def _build_nc_dma_gather():
    """Raw-block variant using InstDMAGatherAnt.

    Index layout (host-prepared, int16): desired[j] = source row for
    gathered slot j, where slot j lands in SBUF dst[j%128, j//128, :].
    The instruction reads index j from idx16[j%16, j//16] (partitions
    0-15, replicated x8 across the 128 partitions for the 8 Q7 cores).
    We want SBUF[p, c] = x[idx_flat[p*T + c]] so the store to DRAM is
    contiguous, i.e. desired = idx_flat.reshape(128, T).T.ravel().
    """
    from contextlib import ExitStack

    assert sum(DG_CHUNKS) == T
    nc = bacc.Bacc(
        "TRN2",
        target_bir_lowering=False,
        debug=False,
        num_devices=N_CORES,
    )
    x = nc.dram_tensor("x", [LENGTH, EMBED], _dt(), kind="ExternalInput").ap()
    idx16 = nc.dram_tensor(
        "idx16", [128, ROWS_PER_CORE // 16], mybir.dt.int16, kind="ExternalInput"
    ).ap()
    out = nc.dram_tensor(
        "out", [128, T * EMBED], _dt(), kind="ExternalOutput"
    ).ap()

    nchunks = len(DG_CHUNKS)
    with ExitStack() as ctx:
        idx_tile = ctx.enter_context(
            nc.sbuf_tensor([128, ROWS_PER_CORE // 16], mybir.dt.int16)
        )
        g = ctx.enter_context(nc.sbuf_tensor([128, T, EMBED], _dt()))
        isem = ctx.enter_context(nc.semaphore("isem"))
        ssem = ctx.enter_context(nc.semaphore("ssem"))
        gsems = [ctx.enter_context(nc.semaphore(f"gsem{i}")) for i in range(nchunks)]
        block = ctx.enter_context(nc.Block())

        @block.scalar
        def _(scalar):
            scalar.dma_start(out=idx_tile[:, :], in_=idx16[:, :]).then_inc(isem, 16)

        @block.gpsimd
        def _(gpsimd):
            gpsimd.wait_ge(isem, 16)
            c0 = 0
            for i, n in enumerate(DG_CHUNKS):
                gpsimd.dma_gather(
                    g[:, c0 : c0 + n, :],
                    x[:, :],
                    idx_tile[:, c0 * 8 : (c0 + n) * 8],
                    n * 128,
                    n * 128,
                    EMBED,
                ).then_inc(gsems[i], 16)
                c0 += n

        @block.sync
        def _(sync):
            c0 = 0
            for i, n in enumerate(DG_CHUNKS):
                sync.wait_ge(gsems[i], 16)
                sync.dma_start(
                    out=out[:, c0 * EMBED : (c0 + n) * EMBED],
                    in_=g[:, c0 : c0 + n, :],
                ).then_inc(ssem, 16)
                c0 += n
            sync.wait_ge(ssem, 16 * nchunks)

    if STRIP_INIT_BARRIER:
        _strip_init_barrier(nc)
    nc.compile()
    return nc


def _build_nc_il():
    """Batch-interleaved gather: x is [LENGTH, IL*EMBED] (IL batches per
    row), each core covers CAP/N_CORES cap positions with one 8 KB
    descriptor per position. IL_OPS ops of [128,1] offsets; store per op."""
    from contextlib import ExitStack

    ilw = INTERLEAVE * EMBED  # elems per interleaved row
    nc = bacc.Bacc(
        "TRN2",
        target_bir_lowering=False,
        debug=False,
        num_devices=N_CORES,
    )
    x = nc.dram_tensor("x", [LENGTH, ilw], _dt(), kind="ExternalInput").ap()
    idx = nc.dram_tensor(
        "idx", [128, IL_OPS], mybir.dt.int32, kind="ExternalInput"
    ).ap()
    out = nc.dram_tensor(
        "out", [128, IL_OPS * ilw], _dt(), kind="ExternalOutput"
    ).ap()

    with ExitStack() as ctx:
        idx_tile = ctx.enter_context(nc.sbuf_tensor([128, IL_OPS], mybir.dt.int32))
        g = ctx.enter_context(nc.sbuf_tensor([128, IL_OPS * ilw], _dt()))
        if WARM_SWDGE:
            warm_tile = ctx.enter_context(nc.sbuf_tensor([128, 2], mybir.dt.int32))
            wsem = ctx.enter_context(nc.semaphore("wsem"))
        isem = ctx.enter_context(nc.semaphore("isem"))
        ssem = ctx.enter_context(nc.semaphore("ssem"))
        gsems = [ctx.enter_context(nc.semaphore(f"gsem{o}")) for o in range(IL_OPS)]
        block = ctx.enter_context(nc.Block())

        @block.scalar
        def _(scalar):
            if IDX_ENGINE == "scalar":
                scalar.dma_start(out=idx_tile[:, :], in_=idx[:, :]).then_inc(
                    isem, 16
                )
            if DUAL_STORE_RING:
                for o in range(1, IL_OPS, 2):
                    scalar.wait_ge(gsems[o], 16)
                    scalar.dma_start(
                        out=out[:, o * ilw : (o + 1) * ilw],
                        in_=g[:, o * ilw : (o + 1) * ilw],
                    ).then_inc(ssem, 16)

        @block.gpsimd
        def _(gpsimd):
            if IDX_ENGINE == "gpsimd":
                gpsimd.dma_start(out=idx_tile[:, :], in_=idx[:, :]).then_inc(
                    isem, 16
                )
            if WARM_SWDGE:
                gpsimd.dma_start(
                    out=warm_tile[:, 1:2], in_=warm_tile[:, 0:1]
                ).then_inc(wsem, 16)
            gpsimd.wait_ge(isem, 16)
            for o in range(IL_OPS):
                gpsimd.indirect_dma_start(
                    out=g[:, o * ilw : (o + 1) * ilw],
                    out_offset=None,
                    in_=x[:, :],
                    in_offset=bass.IndirectOffsetOnAxis(
                        ap=idx_tile[:, o : o + 1], axis=0
                    ),
                ).then_inc(gsems[o], 16)

        @block.sync
        def _(sync):
            for o in range(IL_OPS):
                if DUAL_STORE_RING and o % 2:
                    continue
                sync.wait_ge(gsems[o], 16)
                sync.dma_start(
                    out=out[:, o * ilw : (o + 1) * ilw],
                    in_=g[:, o * ilw : (o + 1) * ilw],
                ).then_inc(ssem, 16)
            sync.wait_ge(ssem, 16 * IL_OPS)

    if STRIP_INIT_BARRIER:
        _strip_init_barrier(nc)
    nc.compile()
    return nc


def _build_nc_raw():
    """Raw blocks, 16 indirect gathers each with a dedicated semaphore so
    nothing couples Q7 emission of op N to DMA completion of earlier ops.
    Stores taper per GGROUPS; store i waits only on the gathers it covers."""
    from contextlib import ExitStack

    nc = bacc.Bacc(
        "TRN2",
        target_bir_lowering=False,
        debug=False,
        num_devices=N_CORES,
    )
    x = nc.dram_tensor("x", [LENGTH, EMBED], _dt(), kind="ExternalInput").ap()
    idx = nc.dram_tensor("idx", [128, T], mybir.dt.int32, kind="ExternalInput").ap()
    out = nc.dram_tensor(
        "out", [128, T * EMBED], _dt(), kind="ExternalOutput"
    ).ap()

    assert sum(GGROUPS) == T
    with ExitStack() as ctx:
        idx_tile = ctx.enter_context(nc.sbuf_tensor([128, T], mybir.dt.int32))
        g = ctx.enter_context(nc.sbuf_tensor([128, T * EMBED], _dt()))
        isem = ctx.enter_context(nc.semaphore("isem"))
        ssem = ctx.enter_context(nc.semaphore("ssem"))
        gsems = [ctx.enter_context(nc.semaphore(f"gsem{t}")) for t in range(T)]
        block = ctx.enter_context(nc.Block())

        @block.scalar
        def _(scalar):
            scalar.dma_start(out=idx_tile[:, :], in_=idx[:, :]).then_inc(isem, 16)

        @block.gpsimd
        def _(gpsimd):
            gpsimd.wait_ge(isem, 16)
            for t in range(T):
                gpsimd.indirect_dma_start(
                    out=g[:, t * EMBED : (t + 1) * EMBED],
                    out_offset=None,
                    in_=x[:, :],
                    in_offset=bass.IndirectOffsetOnAxis(
                        ap=idx_tile[:, t : t + 1], axis=0
                    ),
                ).then_inc(gsems[t], 16)

        @block.sync
        def _(sync):
            t0 = 0
            for gw in GGROUPS:
                for j in range(gw):
                    sync.wait_ge(gsems[t0 + j], 16)
                sync.dma_start(
                    out=out[:, t0 * EMBED : (t0 + gw) * EMBED],
                    in_=g[:, t0 * EMBED : (t0 + gw) * EMBED],
                ).then_inc(ssem, 16)
                t0 += gw
            sync.wait_ge(ssem, 16 * len(GGROUPS))

    if STRIP_INIT_BARRIER:
        _strip_init_barrier(nc)
    nc.compile()
    return nc


def _build_nc():
    if INTERLEAVE > 1:
        return _build_nc_il()
    if USE_DMA_GATHER:
        return _build_nc_dma_gather()
    if USE_RAW:
        return _build_nc_raw()
    nc = bacc.Bacc(
        "TRN2",
        target_bir_lowering=False,
        debug=False,
        num_devices=N_CORES,
    )
    x = nc.dram_tensor("x", [LENGTH, EMBED], _dt(), kind="ExternalInput").ap()
    idx = nc.dram_tensor("idx", [128, T], mybir.dt.int32, kind="ExternalInput").ap()
    out = nc.dram_tensor(
        "out", [128, T * EMBED], _dt(), kind="ExternalOutput"
    ).ap()

    assert sum(GGROUPS) == T

    with tile.TileContext(nc) as tc:
        with (
            tc.tile_pool(name="idxp", bufs=1) as idxp,
            tc.tile_pool(name="io", bufs=len(GGROUPS)) as io,
        ):
            idx_tile = idxp.tile([128, T], mybir.dt.int32)
            nc.scalar.dma_start(out=idx_tile[:], in_=idx[:, :])
            gmax = max(GGROUPS)
            t0 = 0
            for gw in GGROUPS:
                g = io.tile([128, gmax * EMBED], _dt(), tag="g")
                if WIDE:
                    nc.gpsimd.indirect_dma_start(
                        out=g[:, : gw * EMBED],
                        out_offset=None,
                        in_=x[:, :],
                        in_offset=bass.IndirectOffsetOnAxis(
                            ap=idx_tile[:, t0 : t0 + gw], axis=0
                        ),
                    )
                else:
                    for j in range(gw):
                        t = t0 + j
                        nc.gpsimd.indirect_dma_start(
                            out=g[:, j * EMBED : (j + 1) * EMBED],
                            out_offset=None,
                            in_=x[:, :],
                            in_offset=bass.IndirectOffsetOnAxis(
                                ap=idx_tile[:, t : t + 1], axis=0
                            ),
                        )
                nc.sync.dma_start(
                    out=out[:, t0 * EMBED : (t0 + gw) * EMBED],
                    in_=g[:, : gw * EMBED],
                )
                t0 += gw
    if STRIP_INIT_BARRIER:
        _strip_init_barrier(nc)
    nc.compile()
    return nc


def _get_nc():
    global _nc_cache, _nc_cache_key
    key = (
        BF16,
        tuple(GGROUPS),
        WIDE,
        USE_DMA_GATHER,
        tuple(DG_CHUNKS),
        USE_RAW,
        INTERLEAVE,
        IL_OPS,
        STRIP_INIT_BARRIER,
    )
    if _nc_cache is None or _nc_cache_key != key:
        _nc_cache = _build_nc()
        _nc_cache_key = key
    return _nc_cache


def _shard_inputs(inputs: np.ndarray, idx: np.ndarray):
    in_maps = []
    if INTERLEAVE > 1:
        il = INTERLEAVE
        ngroups = B // il  # batch groups; cores split across groups
        cpg = N_CORES // ngroups
        x_ils = [
            np.ascontiguousarray(
                inputs[gi * il : (gi + 1) * il]
                .transpose(1, 0, 2)
                .reshape(LENGTH, il * EMBED)
                .astype(_np_dt())
            )
            for gi in range(ngroups)
        ]
        for k in range(N_CORES):
            gi, q = divmod(k, cpg)
            vals = idx[q * POS_PER_CORE : (q + 1) * POS_PER_CORE].astype(np.int32)
            if SORT_IDX:
                vals = np.sort(vals)
            # slot (p, o) = sorted-rank o*128 + p
            idx_t = np.ascontiguousarray(vals.reshape(IL_OPS, 128).T)
            in_maps.append({"x": x_ils[gi], "idx": idx_t})
        return in_maps
    half = CAP // 2
    for k in range(N_CORES):
        b, h = divmod(k, 2)
        idx_flat = idx[h * half : (h + 1) * half].astype(np.int32)
        xs = np.ascontiguousarray(inputs[b]).astype(_np_dt())
        if USE_DMA_GATHER:
            # desired[j] = row for gathered slot j (slot j -> dst[j%128, j//128])
            desired = idx_flat.reshape(128, T).T.ravel().astype(np.int16)
            # idx16[p, s] = desired[s*16 + p] for p in 0..15, replicated x8
            wrapped = desired.reshape(ROWS_PER_CORE // 16, 16).T  # [16, R/16]
            idx16 = np.ascontiguousarray(np.tile(wrapped, (8, 1)))
            in_maps.append({"x": xs, "idx16": idx16})
        else:
            shard = np.ascontiguousarray(idx_flat.reshape(128, T))
            in_maps.append({"x": xs, "idx": shard})
    return in_maps


def _run(inputs: np.ndarray, idx: np.ndarray, **run_kwargs):
    nc = _get_nc()
    in_maps = _shard_inputs(inputs, idx)
    res = run_bass_kernel_spmd(nc, in_maps, list(range(N_CORES)), **run_kwargs)
    out = np.empty((B, CAP, EMBED), np.float32)
    if INTERLEAVE > 1:
        il = INTERLEAVE
        cpg = N_CORES // (B // il)
        for k in range(N_CORES):
            gi, q = divmod(k, cpg)
            arr = (
                res.results[k]["out"]
                .reshape(128, IL_OPS, il, EMBED)
                .astype(np.float32)
            )
            # [p, o, j, e] -> slot rank o*128+p; rank r holds cap position
            # q*POS + order[r] (order = argsort when SORT_IDX)
            tmp = arr.transpose(2, 1, 0, 3).reshape(il, POS_PER_CORE, EMBED)
            sl = out[
                gi * il : (gi + 1) * il,
                q * POS_PER_CORE : (q + 1) * POS_PER_CORE,
            ]
            if SORT_IDX:
                vals = idx[q * POS_PER_CORE : (q + 1) * POS_PER_CORE]
                sl[:, np.argsort(vals, kind="stable")] = tmp
            else:
                sl[:] = tmp
        return out, res
    half = CAP // 2
    for k in range(N_CORES):
        b, h = divmod(k, 2)
        out[b, h * half : (h + 1) * half] = (
            res.results[k]["out"].reshape(ROWS_PER_CORE, EMBED).astype(np.float32)
        )
    return out, res


def kernel(inputs: np.ndarray, idx: np.ndarray) -> np.ndarray:
    inputs = np.asarray(inputs, dtype=np.float32)
    idx = np.asarray(idx, dtype=np.int32)
    out, _ = _run(inputs, idx)
    return out


# revision 49
# speedup vs baseline: 1.1291x; 1.0598x over previous
"""DropToken gather kernel for Trainium2 (8 NeuronCores).

Computes out[b, c, :] = inputs[b, idx[c], :] (the reference's one-hot
matmul is just a row gather). Memory-bound.

Shipped design (INTERLEAVE=4): ~32.7 us vs 53.7 us f32 baseline.

  * bf16 payload: inputs are cast to bf16 host-side and gathered/stored
    as bf16, halving HBM traffic to 4 MiB read + 4 MiB write per core.
    Output is cast back to f32 host-side. Max elementwise rel err ~2^-9
    (~3e-3), well inside the 2e-2 gate.
  * Batch interleaving: all 4 batches share idx, so x is uploaded as
    [8192, 4*1024] (4 batches concatenated per row) and ONE descriptor
    fetches an 8 KB row covering all batches. Each core covers 512 cap
    positions = 512 descriptors in 4 indirect ops.

    This fixes the two bottlenecks of the per-batch layout at once:
    Q7 SWDGE descriptor emission (~1.4 us per 128-descriptor op; 16 ops
    paced the f32/bf16 kernels at ~22 us) drops to 4 ops (~5.6 us, fully
    hidden), and the random reads grow from 2 KB to 8 KB, lifting the
    measured HBM aggregate to ~373 GB/s (at the per-NC wall).
  * HW semantics (probe-verified): indirect_dma_start uses only ONE
    offset per partition; per-descriptor length = the dest AP's
    per-partition extent. [128, n] offset APs do NOT gather n rows per
    partition (columns past 0 are ignored; consecutive source rows are
    streamed instead) -- hence one [128,1]-offset op per idx column.

Timeline (core 0): ~5.9 us runtime boot + ~2.9 us idx load/receipt +
~22.5 us HBM-bound gather+store (8.39 MB at ~373 GB/s) + ~1.5 us final
store receipt. The data phase sits at the per-NC HBM wall, so further
gains would need a shorter boot or cheaper idx receipt, not DMA work.

Measured dead ends (kept as flags for reference): dma_gather ucode
(InstDMAGatherAnt) pays a ~10.5 us Q7 library reload and ~10 ns/desc;
sorted indices cause HBM bank contention (+2.6 us); dual-ring stores and
gpsimd-issued idx loads are neutral-to-worse; IL=2 (4 KB descs) is ~1 us
worse than IL=4.
"""

import ml_dtypes
import numpy as np

import concourse.bass as bass
import concourse.tile as tile
from concourse import bacc, mybir
from concourse.bass_utils import run_bass_kernel_spmd

B = 4
LENGTH = 8192
EMBED = 1024
CAP = 4096
N_CORES = 8
ROWS_PER_CORE = B * CAP // N_CORES  # 2048
T = ROWS_PER_CORE // 128  # 16 gathered rows per partition

BF16 = True
# Store grouping (in T units): one SBUF tile + one store per group. Early
# groups wide (big store descriptors), tail narrow (short last chain).
GGROUPS = [4, 4, 4, 2, 1, 1]
# WIDE=True issues ONE indirect_dma_start per group with a [128, n] offset
# AP. CoreSim accepts it but HW descriptor ordering differs (wrong results +
# can wedge the device) -- keep False until the HW mapping is understood.
WIDE = False
# InstDMAGatherAnt variant: one Q7 instruction per chunk, but needs a
# ~10.5 us Q7 library reload before the first gather and emits at ~10
# ns/desc anyway -- measured slower (58.5 us) than the indirect path.
USE_DMA_GATHER = False
DG_CHUNKS = [8, 4, 2, 1, 1]
# Raw-block variant of the indirect path: dedicated semaphore per gather
# op (Tile reuses 8 DMASW lanes, which couples op N's emission to op
# N-8's DMA completion and stretches the emission cadence).
USE_RAW = True
# Batch-interleaved gather: upload x as [LENGTH, IL*EMBED] with IL batches
# concatenated per row (all batches share idx), so one descriptor fetches
# IL rows at once. IL=4: 512 descs/core in 4 ops (Q7 emission ~5.6 us,
# fully hidden) and 8 KB random reads (vs 2 KB) for better HBM efficiency.
# HW semantics probe-validated: offsets [128,1], per-desc length = dest
# partition-row bytes.
INTERLEAVE = 4
# cap positions per core and gather ops (one [128,1]-offset op + one store
# per 128 positions) follow from IL: IL=4 -> 512 pos, 4 ops of 1 MB;
# IL=2 -> 1024 pos, 8 ops of 512 KB (smaller tail store, smaller upload).
POS_PER_CORE = CAP * B // (N_CORES * INTERLEAVE)
IL_OPS = POS_PER_CORE // 128
# which engine issues the idx load ("scalar" = HWDGE ACT ring,
# "gpsimd" = SWDGE, starts earlier after boot)
IDX_ENGINE = "scalar"
# Throwaway SWDGE copy on gpsimd to warm the Q7/ring path while idx
# loads: measured worse (median 37.6 vs 32.8) -- the extra ring traffic
# costs more than the warm-up saves.
WARM_SWDGE = False
# Split the LAST idx column into TAIL_SPLIT half-row gather ops via
# element_offset (per-descriptor length follows the dest extent) so the
# final store shrinks. Measured neutral-to-worse (min 33.9 vs 32.8): the
# extra completion boundary eats the smaller-store gain. Keep 1.
TAIL_SPLIT = 1
# Sort each core's indices ascending (slot -> cap position is inverse-
# permuted host-side): SDMA engines then read monotonically increasing
# HBM addresses. Measured WORSE (~+2.6 us) -- clustered addresses create
# HBM bank contention across engines; random spread balances banks.
SORT_IDX = False
# Alternate stores between the SP (sync) and ACT (scalar) HWDGE rings.
# Measured neutral-to-worse (min 34.7 vs 32.8 us); stores are gather-paced
# and the bus is HBM-capped, so a second ring only adds scheduling churn.
DUAL_STORE_RING = False
STRIP_INIT_BARRIER = True

_nc_cache = None
_nc_cache_key = None


def _strip_init_barrier(nc):
    """Remove the Bass-init const memsets and all-engine barrier from the
    entry block. This kernel has no cross-engine deps besides DMA
    semaphores (runtime-zeroed at NEFF load), so engine-boot alignment is
    unnecessary; saves ~3us of startup."""
    blk = nc.m.functions[0].blocks[0]
    blk.instructions = [
        ins
        for ins in blk.instructions
        if not isinstance(
            ins, (mybir.InstMemset, mybir.InstDrain, mybir.InstEventSemaphore)
        )
    ]


def _dt():
    return mybir.dt.bfloat16 if BF16 else mybir.dt.float32


def _np_dt():
    return ml_dtypes.bfloat16 if BF16 else np.float32


def _build_nc_dma_gather():
    """Raw-block variant using InstDMAGatherAnt.

    Index layout (host-prepared, int16): desired[j] = source row for
    gathered slot j, where slot j lands in SBUF dst[j%128, j//128, :].
    The instruction reads index j from idx16[j%16, j//16] (partitions
    0-15, replicated x8 across the 128 partitions for the 8 Q7 cores).
    We want SBUF[p, c] = x[idx_flat[p*T + c]] so the store to DRAM is
    contiguous, i.e. desired = idx_flat.reshape(128, T).T.ravel().
    """
    from contextlib import ExitStack

    assert sum(DG_CHUNKS) == T
    nc = bacc.Bacc(
        "TRN2",
        target_bir_lowering=False,
        debug=False,
        num_devices=N_CORES,
    )
    x = nc.dram_tensor("x", [LENGTH, EMBED], _dt(), kind="ExternalInput").ap()
    idx16 = nc.dram_tensor(
        "idx16", [128, ROWS_PER_CORE // 16], mybir.dt.int16, kind="ExternalInput"
    ).ap()
    out = nc.dram_tensor(
        "out", [128, T * EMBED], _dt(), kind="ExternalOutput"
    ).ap()

    nchunks = len(DG_CHUNKS)
    with ExitStack() as ctx:
        idx_tile = ctx.enter_context(
            nc.sbuf_tensor([128, ROWS_PER_CORE // 16], mybir.dt.int16)
        )
        g = ctx.enter_context(nc.sbuf_tensor([128, T, EMBED], _dt()))
        isem = ctx.enter_context(nc.semaphore("isem"))
        ssem = ctx.enter_context(nc.semaphore("ssem"))
        gsems = [ctx.enter_context(nc.semaphore(f"gsem{i}")) for i in range(nchunks)]
        block = ctx.enter_context(nc.Block())

        @block.scalar
        def _(scalar):
            scalar.dma_start(out=idx_tile[:, :], in_=idx16[:, :]).then_inc(isem, 16)

        @block.gpsimd
        def _(gpsimd):
            gpsimd.wait_ge(isem, 16)
            c0 = 0
            for i, n in enumerate(DG_CHUNKS):
                gpsimd.dma_gather(
                    g[:, c0 : c0 + n, :],
                    x[:, :],
                    idx_tile[:, c0 * 8 : (c0 + n) * 8],
                    n * 128,
                    n * 128,
                    EMBED,
                ).then_inc(gsems[i], 16)
                c0 += n

        @block.sync
        def _(sync):
            c0 = 0
            for i, n in enumerate(DG_CHUNKS):
                sync.wait_ge(gsems[i], 16)
                sync.dma_start(
                    out=out[:, c0 * EMBED : (c0 + n) * EMBED],
                    in_=g[:, c0 : c0 + n, :],
                ).then_inc(ssem, 16)
                c0 += n
            sync.wait_ge(ssem, 16 * nchunks)

    if STRIP_INIT_BARRIER:
        _strip_init_barrier(nc)
    nc.compile()
    return nc


def _build_nc_il():
    """Batch-interleaved gather: x is [LENGTH, IL*EMBED] (IL batches per
    row), each core covers CAP/N_CORES cap positions with one 8 KB
    descriptor per position. IL_OPS ops of [128,1] offsets; store per op."""
    from contextlib import ExitStack

    ilw = INTERLEAVE * EMBED  # elems per interleaved row
    # chunk list: (idx column, elem offset within row, elems) per gather op
    chunks = []
    for o in range(IL_OPS):
        if o == IL_OPS - 1 and TAIL_SPLIT > 1:
            w = ilw // TAIL_SPLIT
            for s in range(TAIL_SPLIT):
                chunks.append((o, s * w, w))
        else:
            chunks.append((o, 0, ilw))
    nc = bacc.Bacc(
        "TRN2",
        target_bir_lowering=False,
        debug=False,
        num_devices=N_CORES,
    )
    x = nc.dram_tensor("x", [LENGTH, ilw], _dt(), kind="ExternalInput").ap()
    idx = nc.dram_tensor(
        "idx", [128, IL_OPS], mybir.dt.int32, kind="ExternalInput"
    ).ap()
    out = nc.dram_tensor(
        "out", [128, IL_OPS * ilw], _dt(), kind="ExternalOutput"
    ).ap()

    with ExitStack() as ctx:
        idx_tile = ctx.enter_context(nc.sbuf_tensor([128, IL_OPS], mybir.dt.int32))
        g = ctx.enter_context(nc.sbuf_tensor([128, IL_OPS * ilw], _dt()))
        if WARM_SWDGE:
            warm_tile = ctx.enter_context(nc.sbuf_tensor([128, 2], mybir.dt.int32))
            wsem = ctx.enter_context(nc.semaphore("wsem"))
        isem = ctx.enter_context(nc.semaphore("isem"))
        ssem = ctx.enter_context(nc.semaphore("ssem"))
        gsems = [
            ctx.enter_context(nc.semaphore(f"gsem{i}")) for i in range(len(chunks))
        ]
        block = ctx.enter_context(nc.Block())

        @block.scalar
        def _(scalar):
            if IDX_ENGINE == "scalar":
                scalar.dma_start(out=idx_tile[:, :], in_=idx[:, :]).then_inc(
                    isem, 16
                )
            if DUAL_STORE_RING:
                for i, (o, eo, w) in enumerate(chunks):
                    if i % 2 == 0:
                        continue
                    scalar.wait_ge(gsems[i], 16)
                    scalar.dma_start(
                        out=out[:, o * ilw + eo : o * ilw + eo + w],
                        in_=g[:, o * ilw + eo : o * ilw + eo + w],
                    ).then_inc(ssem, 16)

        @block.gpsimd
        def _(gpsimd):
            if IDX_ENGINE == "gpsimd":
                gpsimd.dma_start(out=idx_tile[:, :], in_=idx[:, :]).then_inc(
                    isem, 16
                )
            if WARM_SWDGE:
                gpsimd.dma_start(
                    out=warm_tile[:, 1:2], in_=warm_tile[:, 0:1]
                ).then_inc(wsem, 16)
            gpsimd.wait_ge(isem, 16)
            for i, (o, eo, w) in enumerate(chunks):
                gpsimd.indirect_dma_start(
                    out=g[:, o * ilw + eo : o * ilw + eo + w],
                    out_offset=None,
                    in_=x[:, :],
                    in_offset=bass.IndirectOffsetOnAxis(
                        ap=idx_tile[:, o : o + 1], axis=0
                    ),
                    element_offset=eo,
                ).then_inc(gsems[i], 16)

        @block.sync
        def _(sync):
            for i, (o, eo, w) in enumerate(chunks):
                if DUAL_STORE_RING and i % 2:
                    continue
                sync.wait_ge(gsems[i], 16)
                sync.dma_start(
                    out=out[:, o * ilw + eo : o * ilw + eo + w],
                    in_=g[:, o * ilw + eo : o * ilw + eo + w],
                ).then_inc(ssem, 16)
            sync.wait_ge(ssem, 16 * len(chunks))

    if STRIP_INIT_BARRIER:
        _strip_init_barrier(nc)
    nc.compile()
    return nc


def _build_nc_raw():
    """Raw blocks, 16 indirect gathers each with a dedicated semaphore so
    nothing couples Q7 emission of op N to DMA completion of earlier ops.
    Stores taper per GGROUPS; store i waits only on the gathers it covers."""
    from contextlib import ExitStack

    nc = bacc.Bacc(
        "TRN2",
        target_bir_lowering=False,
        debug=False,
        num_devices=N_CORES,
    )
    x = nc.dram_tensor("x", [LENGTH, EMBED], _dt(), kind="ExternalInput").ap()
    idx = nc.dram_tensor("idx", [128, T], mybir.dt.int32, kind="ExternalInput").ap()
    out = nc.dram_tensor(
        "out", [128, T * EMBED], _dt(), kind="ExternalOutput"
    ).ap()

    assert sum(GGROUPS) == T
    with ExitStack() as ctx:
        idx_tile = ctx.enter_context(nc.sbuf_tensor([128, T], mybir.dt.int32))
        g = ctx.enter_context(nc.sbuf_tensor([128, T * EMBED], _dt()))
        isem = ctx.enter_context(nc.semaphore("isem"))
        ssem = ctx.enter_context(nc.semaphore("ssem"))
        gsems = [ctx.enter_context(nc.semaphore(f"gsem{t}")) for t in range(T)]
        block = ctx.enter_context(nc.Block())

        @block.scalar
        def _(scalar):
            scalar.dma_start(out=idx_tile[:, :], in_=idx[:, :]).then_inc(isem, 16)

        @block.gpsimd
        def _(gpsimd):
            gpsimd.wait_ge(isem, 16)
            for t in range(T):
                gpsimd.indirect_dma_start(
                    out=g[:, t * EMBED : (t + 1) * EMBED],
                    out_offset=None,
                    in_=x[:, :],
                    in_offset=bass.IndirectOffsetOnAxis(
                        ap=idx_tile[:, t : t + 1], axis=0
                    ),
                ).then_inc(gsems[t], 16)

        @block.sync
        def _(sync):
            t0 = 0
            for gw in GGROUPS:
                for j in range(gw):
                    sync.wait_ge(gsems[t0 + j], 16)
                sync.dma_start(
                    out=out[:, t0 * EMBED : (t0 + gw) * EMBED],
                    in_=g[:, t0 * EMBED : (t0 + gw) * EMBED],
                ).then_inc(ssem, 16)
                t0 += gw
            sync.wait_ge(ssem, 16 * len(GGROUPS))

    if STRIP_INIT_BARRIER:
        _strip_init_barrier(nc)
    nc.compile()
    return nc


def _build_nc():
    if INTERLEAVE > 1:
        return _build_nc_il()
    if USE_DMA_GATHER:
        return _build_nc_dma_gather()
    if USE_RAW:
        return _build_nc_raw()
    nc = bacc.Bacc(
        "TRN2",
        target_bir_lowering=False,
        debug=False,
        num_devices=N_CORES,
    )
    x = nc.dram_tensor("x", [LENGTH, EMBED], _dt(), kind="ExternalInput").ap()
    idx = nc.dram_tensor("idx", [128, T], mybir.dt.int32, kind="ExternalInput").ap()
    out = nc.dram_tensor(
        "out", [128, T * EMBED], _dt(), kind="ExternalOutput"
    ).ap()

    assert sum(GGROUPS) == T

    with tile.TileContext(nc) as tc:
        with (
            tc.tile_pool(name="idxp", bufs=1) as idxp,
            tc.tile_pool(name="io", bufs=len(GGROUPS)) as io,
        ):
            idx_tile = idxp.tile([128, T], mybir.dt.int32)
            nc.scalar.dma_start(out=idx_tile[:], in_=idx[:, :])
            gmax = max(GGROUPS)
            t0 = 0
            for gw in GGROUPS:
                g = io.tile([128, gmax * EMBED], _dt(), tag="g")
                if WIDE:
                    nc.gpsimd.indirect_dma_start(
                        out=g[:, : gw * EMBED],
                        out_offset=None,
                        in_=x[:, :],
                        in_offset=bass.IndirectOffsetOnAxis(
                            ap=idx_tile[:, t0 : t0 + gw], axis=0
                        ),
                    )
                else:
                    for j in range(gw):
                        t = t0 + j
                        nc.gpsimd.indirect_dma_start(
                            out=g[:, j * EMBED : (j + 1) * EMBED],
                            out_offset=None,
                            in_=x[:, :],
                            in_offset=bass.IndirectOffsetOnAxis(
                                ap=idx_tile[:, t : t + 1], axis=0
                            ),
                        )
                nc.sync.dma_start(
                    out=out[:, t0 * EMBED : (t0 + gw) * EMBED],
                    in_=g[:, : gw * EMBED],
                )
                t0 += gw
    if STRIP_INIT_BARRIER:
        _strip_init_barrier(nc)
    nc.compile()
    return nc


def _get_nc():
    global _nc_cache, _nc_cache_key
    key = (
        BF16,
        tuple(GGROUPS),
        WIDE,
        USE_DMA_GATHER,
        tuple(DG_CHUNKS),
        USE_RAW,
        INTERLEAVE,
        IL_OPS,
        TAIL_SPLIT,
        STRIP_INIT_BARRIER,
    )
    if _nc_cache is None or _nc_cache_key != key:
        _nc_cache = _build_nc()
        _nc_cache_key = key
    return _nc_cache


def _shard_inputs(inputs: np.ndarray, idx: np.ndarray):
    in_maps = []
    if INTERLEAVE > 1:
        il = INTERLEAVE
        ngroups = B // il  # batch groups; cores split across groups
        cpg = N_CORES // ngroups
        x_ils = [
            np.ascontiguousarray(
                inputs[gi * il : (gi + 1) * il]
                .transpose(1, 0, 2)
                .reshape(LENGTH, il * EMBED)
                .astype(_np_dt())
            )
            for gi in range(ngroups)
        ]
        for k in range(N_CORES):
            gi, q = divmod(k, cpg)
            vals = idx[q * POS_PER_CORE : (q + 1) * POS_PER_CORE].astype(np.int32)
            if SORT_IDX:
                vals = np.sort(vals)
            # slot (p, o) = sorted-rank o*128 + p
            idx_t = np.ascontiguousarray(vals.reshape(IL_OPS, 128).T)
            in_maps.append({"x": x_ils[gi], "idx": idx_t})
        return in_maps
    half = CAP // 2
    for k in range(N_CORES):
        b, h = divmod(k, 2)
        idx_flat = idx[h * half : (h + 1) * half].astype(np.int32)
        xs = np.ascontiguousarray(inputs[b]).astype(_np_dt())
        if USE_DMA_GATHER:
            # desired[j] = row for gathered slot j (slot j -> dst[j%128, j//128])
            desired = idx_flat.reshape(128, T).T.ravel().astype(np.int16)
            # idx16[p, s] = desired[s*16 + p] for p in 0..15, replicated x8
            wrapped = desired.reshape(ROWS_PER_CORE // 16, 16).T  # [16, R/16]
            idx16 = np.ascontiguousarray(np.tile(wrapped, (8, 1)))
            in_maps.append({"x": xs, "idx16": idx16})
        else:
            shard = np.ascontiguousarray(idx_flat.reshape(128, T))
            in_maps.append({"x": xs, "idx": shard})
    return in_maps


def _run(inputs: np.ndarray, idx: np.ndarray, **run_kwargs):
    nc = _get_nc()
    in_maps = _shard_inputs(inputs, idx)
    res = run_bass_kernel_spmd(nc, in_maps, list(range(N_CORES)), **run_kwargs)
    out = np.empty((B, CAP, EMBED), np.float32)
    if INTERLEAVE > 1:
        il = INTERLEAVE
        cpg = N_CORES // (B // il)
        for k in range(N_CORES):
            gi, q = divmod(k, cpg)
            arr = (
                res.results[k]["out"]
                .reshape(128, IL_OPS, il, EMBED)
                .astype(np.float32)
            )
            # [p, o, j, e] -> slot rank o*128+p; rank r holds cap position
            # q*POS + order[r] (order = argsort when SORT_IDX)
            tmp = arr.transpose(2, 1, 0, 3).reshape(il, POS_PER_CORE, EMBED)
            sl = out[
                gi * il : (gi + 1) * il,
                q * POS_PER_CORE : (q + 1) * POS_PER_CORE,
            ]
            if SORT_IDX:
                vals = idx[q * POS_PER_CORE : (q + 1) * POS_PER_CORE]
                sl[:, np.argsort(vals, kind="stable")] = tmp
            else:
                sl[:] = tmp
        return out, res
    half = CAP // 2
    for k in range(N_CORES):
        b, h = divmod(k, 2)
        out[b, h * half : (h + 1) * half] = (
            res.results[k]["out"].reshape(ROWS_PER_CORE, EMBED).astype(np.float32)
        )
    return out, res


def kernel(inputs: np.ndarray, idx: np.ndarray) -> np.ndarray:
    inputs = np.asarray(inputs, dtype=np.float32)
    idx = np.asarray(idx, dtype=np.int32)
    out, _ = _run(inputs, idx)
    return out
